# revision 1
# baseline (speedup 1.0000x reference)
"""RWKV-style AttentionBlock kernel for 8 Trainium2 NeuronCores.

Problem: B=8, T=4096, D=1024, f32 in/out.
  per sequence: k/v/r = token-shift-mixed x @ W{k,v,r}.T ; imp = exp(k)
  WKV linear recurrence over time (per-channel decay), bonus-gain readout,
  rwkv = sigmoid(r) * wkv ; out = rwkv @ Wo.T

Sharding: pure data-parallel, one batch element per core (no collectives).

Layout strategy (everything channel-major so the time recurrence runs on the
DVE's native tensor_tensor_scan along the free axis):
  - host pre-transposes x -> x^T [D, T] packed as [128, 8, T] fp16
  - host pre-transposes weights -> W.T packed [128(e), 8(e-tiles), 1024(c)] fp16
  - GEMMs: out^T[c,t] = sum_e W.T[e,c] * x^T[e,t] via PE (contraction on
    partitions), fp16 operands, f32 PSUM accumulation
  - scan: state = decay*state + u, per-channel (partition), time on free axis
  - sigmoid and the WKV division are fused into one reciprocal:
      rwkv = num * exp(-ln((1+exp(-r)) * den))   (ACT does only Exp/Ln -> one
      activation table set, no table thrashing)
  - output dx^T written fp16, host transposes back and upcasts to f32.
"""

import os
import numpy as np
from contextlib import ExitStack

import concourse.bass as bass
import concourse.mybir as mybir
import concourse.tile as tile
from concourse import bacc
from concourse.bass_utils import run_bass_kernel_spmd

# ---------------------------------------------------------------------------
# The bacc act-table-load pass picks, per activation, a function set from
# get_activation_tables(); with the default tables Exp resolves to
# "exp_and_others" and Ln to "natural_log", so a kernel alternating Exp/Ln
# reloads the ACT tables (~1.3us each) twice per tile. Restrict Exp/Ln
# membership to the one set that has both ("natural_log_exp_and_others",
# same index, so emitted act_func_set_ids stay valid) -> single table load.
import concourse.hw_specs as _hw_specs

_orig_get_activation_tables = _hw_specs.get_activation_tables


def _pinned_activation_tables(arch):
    tabs = _orig_get_activation_tables(arch)
    AF_ = mybir.ActivationFunctionType
    both = [n for n, fs in tabs.items() if AF_.Exp in fs and AF_.Ln in fs]
    if both:
        keep = both[0]
        for n, fs in tabs.items():
            if n != keep:
                fs.discard(AF_.Exp)
                fs.discard(AF_.Ln)
    return tabs


if os.environ.get("PIN_ACT_TABLES", "1") == "1":
    _hw_specs.get_activation_tables = _pinned_activation_tables
    bacc.get_activation_tables = _pinned_activation_tables

P = 128
D = 1024
DT = D // P          # 8 channel tiles
B = 8
T_FULL = 4096
TC_DEFAULT = 512

F16 = mybir.dt.float16
F32 = mybir.dt.float32
# NOTE: fp16 per-partition scalar operands (scalar_tensor_tensor / broadcast
# scan data0) deadlock the DVE on hardware -- params must stay f32.
PPDT = F32
# gpsimd shares its SBUF port with the DVE: offloading elementwise ops
# there just steals DVE bandwidth (measured stt 734 -> 1583ns). Keep off.
GPS = os.environ.get("GPSIMD_OFFLOAD", "0") == "1"
AL = mybir.AluOpType
AF = mybir.ActivationFunctionType


def build(T=T_FULL, TC=TC_DEFAULT):
    """Build the single-core Bass graph (SPMD across cores via run_bass_kernel_spmd)."""
    assert T % TC == 0
    NCH = T // TC
    nc = bacc.Bacc("TRN2", target_bir_lowering=False, debug=False, num_devices=B)

    x_d = nc.dram_tensor("x", [P, DT, T], F16, kind="ExternalInput")
    wk_d = nc.dram_tensor("wk", [P, DT, D], F16, kind="ExternalInput")
    wv_d = nc.dram_tensor("wv", [P, DT, D], F16, kind="ExternalInput")
    wr_d = nc.dram_tensor("wr", [P, DT, D], F16, kind="ExternalInput")
    wo_d = nc.dram_tensor("wo", [P, DT, D], F16, kind="ExternalInput")
    # per-channel params, packed [128, DT, 8]: mix_k, mix_v, mix_r, decay, gain
    pp_d = nc.dram_tensor("pp", [P, DT, 8], PPDT, kind="ExternalInput")
    out_d = nc.dram_tensor("out", [P, DT, T], F16, kind="ExternalOutput")

    with tile.TileContext(nc) as tc, ExitStack() as ctx:
        const = ctx.enter_context(tc.tile_pool(name="const", bufs=1))
        xpool = ctx.enter_context(tc.tile_pool(name="xpool", bufs=2))
        mixp = ctx.enter_context(tc.tile_pool(name="mixp", bufs=2))
        rwkvp = ctx.enter_context(tc.tile_pool(name="rwkvp", bufs=2))
        scanp = ctx.enter_context(tc.tile_pool(name="scanp", bufs=2))
        tmpp = ctx.enter_context(tc.tile_pool(name="tmpp", bufs=2))
        diffp = ctx.enter_context(tc.tile_pool(name="diffp", bufs=1))
        outp = ctx.enter_context(tc.tile_pool(name="outp", bufs=1))
        psp = ctx.enter_context(tc.tile_pool(name="psp", bufs=5, space="PSUM"))
        pso = ctx.enter_context(tc.tile_pool(name="pso", bufs=3, space="PSUM"))

        # ---- params + first x chunk first (so DVE/PE can start while the
        # larger weight DMAs stream in), then resident weights
        pp_sb = const.tile([P, DT, 8], PPDT, tag="pp")
        nc.sync.dma_start(pp_sb[:], pp_d[:])
        xt0 = xpool.tile([P, DT, TC + 1], F16, tag="xt", name="xt0")
        for dt_i in range(DT):
            nc.vector.memset(xt0[:, dt_i, 0:1], 0.0)
        nc.sync.dma_start(xt0[:, :, 1:], x_d[:, :, 0:TC])
        w_sb = {}
        for nm, dram in (("k", wk_d), ("v", wv_d), ("r", wr_d), ("o", wo_d)):
            w = const.tile([P, DT, D], F16, tag=f"w{nm}")
            nc.sync.dma_start(w[:], dram[:])
            w_sb[nm] = w

        def pc(dt_i, j):
            return pp_sb[:, dt_i, j : j + 1]

        prev_c = [None] * DT
        prev_n = [None] * DT

        for ch in range(NCH):
            t0 = ch * TC
            # ---- x chunk with one column of history at index 0
            if ch == 0:
                xt = xt0
            else:
                xt = xpool.tile([P, DT, TC + 1], F16, tag="xt")
                nc.sync.dma_start(xt[:], x_d[:, :, t0 - 1 : t0 + TC])

            # ---- token-shift mixing (fp16, DVE)
            xk = mixp.tile([P, DT, TC], F16, tag="xk")
            xv = mixp.tile([P, DT, TC], F16, tag="xv")
            xr = mixp.tile([P, DT, TC], F16, tag="xr")
            diffs = []
            for dt_i in range(DT):
                diff = diffp.tile([P, TC], F16, tag=f"diff{dt_i}")
                (nc.gpsimd if GPS else nc.vector).tensor_sub(
                    diff[:], xt[:, dt_i, 1:], xt[:, dt_i, 0:TC]
                )
                diffs.append(diff)
            # all xk mixes first: the K projection (and the serial scan chain
            # behind it) is the critical path; xv/xr overlap with K GEMMs
            for j, dest in ((0, xk), (1, xv), (2, xr)):
                for dt_i in range(DT):
                    nc.vector.scalar_tensor_tensor(
                        dest[:, dt_i, :], diffs[dt_i][:], pc(dt_i, j),
                        xt[:, dt_i, 0:TC], AL.mult, AL.add
                    )

            rwkv = rwkvp.tile([P, DT, TC], F16, tag="rwkv")

            for dt_i in range(DT):
                cs = slice(dt_i * P, (dt_i + 1) * P)
                # K projection -> imp = exp(k)
                ps_k = psp.tile([P, TC], F32, tag="ps")
                for eo in range(DT):
                    nc.tensor.matmul(
                        ps_k[:], w_sb["k"][:, eo, cs], xk[:, eo, :],
                        start=(eo == 0), stop=(eo == DT - 1),
                    )
                imp = tmpp.tile([P, TC], F16, tag="imp")
                nc.scalar.activation(imp[:], ps_k[:], AF.Exp)

                # V projection -> v16
                ps_v = psp.tile([P, TC], F32, tag="ps")
                for eo in range(DT):
                    nc.tensor.matmul(
                        ps_v[:], w_sb["v"][:, eo, cs], xv[:, eo, :],
                        start=(eo == 0), stop=(eo == DT - 1),
                    )
                v16 = tmpp.tile([P, TC], F16, tag="v16")
                nc.scalar.copy(v16[:], ps_v[:])

                # R projection -> er = exp(-r)
                ps_r = psp.tile([P, TC], F32, tag="ps")
                for eo in range(DT):
                    nc.tensor.matmul(
                        ps_r[:], w_sb["r"][:, eo, cs], xr[:, eo, :],
                        start=(eo == 0), stop=(eo == DT - 1),
                    )
                er = tmpp.tile([P, TC], F16, tag="er")
                nc.scalar.activation(er[:], ps_r[:], AF.Exp, scale=-1.0)

                # u = imp * v
                u = tmpp.tile([P, TC], F16, tag="u")
                (nc.gpsimd if GPS else nc.vector).tensor_mul(u[:], imp[:], v16[:])

                # WKV scans (f32 state inside the DVE, fp16 stored)
                c_sc = scanp.tile([P, TC], F16, tag=f"c{dt_i}")
                n_sc = scanp.tile([P, TC], F16, tag=f"n{dt_i}")
                decay_b = pc(dt_i, 3).to_broadcast((P, TC))
                init_c = 0.0 if ch == 0 else prev_c[dt_i][:, TC - 1 : TC]
                init_n = 0.0 if ch == 0 else prev_n[dt_i][:, TC - 1 : TC]
                nc.vector.tensor_tensor_scan(
                    c_sc[:], decay_b, u[:], init_c, AL.mult, AL.add
                )
                nc.vector.tensor_tensor_scan(
                    n_sc[:], decay_b, imp[:], init_n, AL.mult, AL.add
                )
                prev_c[dt_i] = c_sc
                prev_n[dt_i] = n_sc

                # num = c + gain*u ; den = n + gain*imp
                num = tmpp.tile([P, TC], F16, tag="num")
                nc.vector.scalar_tensor_tensor(
                    num[:], u[:], pc(dt_i, 4), c_sc[:], AL.mult, AL.add
                )
                den = tmpp.tile([P, TC], F16, tag="den")
                nc.vector.scalar_tensor_tensor(
                    den[:], imp[:], pc(dt_i, 4), n_sc[:], AL.mult, AL.add
                )
                # den2 = (1 + er) * den ; rec = exp(-ln(den2)) on ACT
                den2 = tmpp.tile([P, TC], F16, tag="den2")
                nc.vector.scalar_tensor_tensor(
                    den2[:], er[:], 1.0, den[:], AL.add, AL.mult
                )
                # in-place on ACT: den2 -> ln(den2) -> exp(-ln(den2)) = 1/den2
                nc.scalar.activation(den2[:], den2[:], AF.Ln)
                nc.scalar.activation(den2[:], den2[:], AF.Exp, scale=-1.0)
                nc.vector.tensor_mul(rwkv[:, dt_i, :], num[:], den2[:])

            # ---- output projection dx^T = Wo^T-contraction over channels
            out16 = outp.tile([P, DT, TC], F16, tag="out16")
            for co in range(DT):
                cs = slice(co * P, (co + 1) * P)
                ps_o = pso.tile([P, TC], F32, tag="pso")
                for eo in range(DT):
                    nc.tensor.matmul(
                        ps_o[:], w_sb["o"][:, eo, cs], rwkv[:, eo, :],
                        start=(eo == 0), stop=(eo == DT - 1),
                    )
                nc.scalar.copy(out16[:, co, :], ps_o[:])
            nc.sync.dma_start(out_d[:, :, t0 : t0 + TC], out16[:])

    nc.compile()
    return nc


def _pack_vec(v):
    # [D] -> [P, DT]
    return np.ascontiguousarray(v.reshape(DT, P).T)


def pack_inputs(x, Wk, Wv, Wr, Wo, mix_k, mix_v, mix_r, log_gain, log_decay):
    """Host-side sharding + relayout. Returns per-core in_maps."""
    T = x.shape[1]
    decay = np.exp(-np.exp(log_decay.astype(np.float64))).astype(np.float32)
    gain = (np.exp(log_gain.astype(np.float64)) - 1.0).astype(np.float32)
    pp = np.zeros((P, DT, 8), np.float32)
    for j, v in enumerate((mix_k, mix_v, mix_r, decay, gain)):
        pp[:, :, j] = _pack_vec(v.astype(np.float32))

    def packw(W):
        # W [c, e] -> W.T [e, c] -> [P, DT, D] fp16
        return np.ascontiguousarray(
            W.T.reshape(DT, P, D).transpose(1, 0, 2)
        ).astype(np.float16)

    wks, wvs, wrs, wos = packw(Wk), packw(Wv), packw(Wr), packw(Wo)
    in_maps = []
    for b in range(x.shape[0]):
        xb = np.ascontiguousarray(
            x[b].T.reshape(DT, P, T).transpose(1, 0, 2)
        ).astype(np.float16)
        in_maps.append(
            {"x": xb, "wk": wks, "wv": wvs, "wr": wrs, "wo": wos, "pp": pp}
        )
    return in_maps


def unpack_output(arrs, T):
    # list of [P, DT, T] fp16 -> [B, T, D] f32
    out = np.empty((len(arrs), T, D), np.float32)
    for b, a in enumerate(arrs):
        out[b] = a.astype(np.float32).transpose(2, 1, 0).reshape(T, D)
    return out


_NC_CACHE = {}


def run(inputs, trace=False, **kw):
    x = np.asarray(inputs["x"])
    Bx, T, Dx = x.shape
    assert Dx == D and Bx == B
    key = (T, TC_DEFAULT)
    if key not in _NC_CACHE:
        _NC_CACHE[key] = build(T=T)
    nc = _NC_CACHE[key]
    in_maps = pack_inputs(
        x,
        np.asarray(inputs["Wk"]), np.asarray(inputs["Wv"]),
        np.asarray(inputs["Wr"]), np.asarray(inputs["Wo"]),
        np.asarray(inputs["mix_k"]), np.asarray(inputs["mix_v"]),
        np.asarray(inputs["mix_r"]),
        np.asarray(inputs["log_gain"]), np.asarray(inputs["log_decay"]),
    )
    res = run_bass_kernel_spmd(nc, in_maps, core_ids=list(range(B)), trace=trace, **kw)
    out = unpack_output([res.results[i]["out"] for i in range(B)], T)
    return out, res


def kernel(**inputs):
    return run(inputs)[0]


if __name__ == "__main__":
    nc = build(T=512)
    print("built ok")

